# revision 26
# baseline (speedup 1.0000x reference)
"""Trainium2 Bass kernel for nn_MixedLinear_89979564851799.

The reference computes y = x @ W.T where W is the block-dequantized weight;
setup_inputs() ships the module's precomputed dequantized transposed weight
w_t (IN, OUT), so y == x @ w_t up to fp32 matmul reassociation.  The kernel
runs a single 8192x4096x4096 matmul, data-parallel over tokens across 8
NeuronCores.

Numerics: mixed bf16 / fp8-DoubleRow.  The last K8 = 256*N8 of the
contraction runs as fp8e4 DoubleRow matmuls (2 k-tiles per MM at the same
216ns issue gap as one bf16 MM -> 2x throughput on that span; measured
with probe_doublerow.py on this part).  The fp8 range covers the module's
fp8-quantized weight partition (k in [3584,4096), whose dequantized values
are EXACTLY representable in TRN fp8e4 under a power-2 scale) plus
256*(N8-2) columns of the fp4 partition (e4m3 rounding error ~2.4% rms on
that slice).  x is e4m3 on the fp8 range.  CPU simulation of the exact
scheme on the reference data: rel err 9.6e-3 (N8=2) / 1.34e-2 (N8=3) /
1.63e-2 (N8=4) vs the 2e-2 gate; bf16-only measures 2.26e-3.  Measured HW
rel err matches the simulation to 5 digits (1.632e-2 at N8=4).  N8=5
(1.88e-2) leaves too little margin even with per-column scale tricks.

Scale handling: fp8 operands need power-2 scaling (x*2^a, w*2^b) to sit in
e4m3 range; the bf16 operands are pre-scaled by the same powers (exact in
bf16) so both matmul flavors accumulate into one PSUM group, and the
psum->sbuf copy applies 2^-(a+b) (tensor_scalar_mul, same cost as the
plain copy).

Schedule (from ntff profiles of each round): interleaved per-k-tile DMA
descriptors in consumption order; n-chunk 0 k-OUTER across 8 m-tiles / 8
psum banks (PE starts ~11us in and never starves: demand 222GB/s < the
~300GB/s 16 DMA engines deliver); n-chunks 1-7 m-inner with staggered psum
copies; dummy N=64 matmuls pad the DMA head so the PE's HAM clock gate is
warm (2.4GHz) when real work starts; activation table warmed early; the
last n-chunk's stores spread across both hw DMA queues and the final
group's copy split across DVE+ACT to shrink the serial tail.

History on this part: baseline bf16 469.5us -> DMA/schedule fixes 464.3
-> +fp8 DoubleRow 408.1 -> head/tail tuning ~405.2us (PE roofline for the
1792-MM stream is ~387us; the rest is fixed preamble ~7.5us, DMA head,
~7us of LDWEIGHTS-reorder hiccups, and the drain tail).
"""

import os
import numpy as np
import ml_dtypes

P = 128
TOKENS, IN, OUT = 8192, 4096, 4096
NCORES = 8
M_PER_CORE = TOKENS // NCORES      # 1024
KT = IN // P                       # 32 k-tiles
MT = M_PER_CORE // P               # 8 m-tiles
NCH = 8                            # n chunks
NW = OUT // NCH                    # 512 cols per chunk (= 1 PSUM bank fp32)

N8 = 4                             # DoubleRow 256-k blocks (fp8 span = 256*N8)
KTB = KT - 2 * N8                  # bf16 k-tiles
KSPLIT = KTB * P                   # k index where the fp8 span starts
GS = KTB // 2                      # bf16 w chunk size (k-tiles) for nch 1-7

FP8_MAX = 240.0                    # TRN fp8e4 max normal

# Per-group matmul emission order.  Toggling DoubleRow mode costs ~190ns
# (measured: fully interleaving DR among bf16 MMs added ~84us = ~8
# transitions/group), so DR blocks stay clustered at one end, and the
# m-inner groups snake (alternate groups run DR-first) so consecutive
# groups' DR clusters abut: ~1 mode transition per group instead of 2.
MM_SEQ = [("b", _kt) for _kt in range(KTB)] + [("d", _j) for _j in range(N8)]

# Results of the traced run (exec_time_ns etc.) for test harnesses.
LAST_RESULT = None
_BUILT = {}


def _patch_tile_drain():
    """The walrus build in this container rejects instructions carrying more
    than one sync-wait (CoreV3GenImpl setupSyncWait: "Too many sync wait
    commands").  Tile's scheduler freely assigns several waits to one
    instruction, so (a) wrap _commit_instruction to hoist extra waits onto
    single-wait NOPs on the same engine just before the offender, and
    (b) split the kernel-tail Drain (which collects one wait per DMA queue)
    into a chain of single-wait Drains."""
    import concourse.tile as tile_mod
    import concourse.mybir as mybir
    import bass_rust
    from concourse.vector_clock import ScopedClock

    if getattr(tile_mod.TileContext, "_single_wait_drain_patch", False):
        return

    orig_commit = tile_mod.TileContext._commit_instruction

    def _commit_instruction(self, inst, lazy_reg_writes=True):
        si = getattr(inst, "sync_info", None)
        if (
            si is not None
            and len(si.on_wait) > 1
            and inst.engine != mybir.EngineType.Unassigned
        ):
            waits = list(si.on_wait)
            for w in waits[:-1]:
                nop = mybir.InstNoOp(
                    name=self.nc.get_next_instruction_name(),
                    engine=inst.engine,
                    sync_info=mybir.SyncInfo(on_wait=[w], on_update=[]),
                    bass_nofuse=True,
                )
                orig_commit(self, nop, lazy_reg_writes=False)
            inst.sync_info = mybir.SyncInfo(
                on_wait=[waits[-1]], on_update=list(si.on_update)
            )
        return orig_commit(self, inst, lazy_reg_writes)

    tile_mod.TileContext._commit_instruction = _commit_instruction

    def _drain_and_barrier(self, tick_clock, wait_clock):
        drain_inst = self.nc.sync.drain()
        wait_clock.add_sem_waits(
            drain_inst.ins, ScopedClock({None: tick_clock.global_clock})
        )
        si = drain_inst.ins.sync_info
        if si is not None and len(si.on_wait) > 1:
            waits = list(si.on_wait)
            drain_inst.ins.sync_info = bass_rust.SyncInfo(
                on_wait=[waits[0]], on_update=list(si.on_update)
            )
            for w in waits[1:]:
                extra = self.nc.sync.drain()
                extra.ins.sync_info = bass_rust.SyncInfo(on_wait=[w], on_update=[])
        self.nc.all_engine_barrier()
        popped = self.nc._tile_sem_poison_stack.pop()
        assert popped is self._sem_poison
        self.nc.clear_and_free_semaphores(list(self.sems.allocated().values()))
        self.nc.all_engine_barrier()

    tile_mod.TileContext._drain_and_barrier = _drain_and_barrier
    tile_mod.TileContext._single_wait_drain_patch = True


def _build(descale):
    """descale = 2^-(a+b), baked into the psum->sbuf copies."""
    if descale in _BUILT:
        return _BUILT[descale]
    import concourse.bass as bass
    import concourse.tile as tile
    from concourse import mybir

    _patch_tile_drain()

    nc = bass.Bass("TRN2", debug=False)
    xb_d = nc.dram_tensor(
        "xb", [KTB, P, M_PER_CORE], mybir.dt.bfloat16, kind="ExternalInput"
    ).ap()
    x8_d = nc.dram_tensor(
        "x8", [N8, P, 2, M_PER_CORE], mybir.dt.float8e4, kind="ExternalInput"
    ).ap()
    # n-chunk 0 of w, per-k-tile descriptors
    wb0_d = nc.dram_tensor(
        "wb0", [KTB, P, NW], mybir.dt.bfloat16, kind="ExternalInput"
    ).ap()
    w80_d = nc.dram_tensor(
        "w80", [N8, P, 2, NW], mybir.dt.float8e4, kind="ExternalInput"
    ).ap()
    # n-chunks 1-7: bf16 in two GS-k-tile chunks, fp8 in one block
    wbr_d = nc.dram_tensor(
        "wbr", [NCH - 1, 2, P, GS, NW], mybir.dt.bfloat16, kind="ExternalInput"
    ).ap()
    w8r_d = nc.dram_tensor(
        "w8r", [NCH - 1, P, N8, 2, NW], mybir.dt.float8e4, kind="ExternalInput"
    ).ap()
    y_d = nc.dram_tensor(
        "y", [M_PER_CORE, OUT], mybir.dt.float32, kind="ExternalOutput"
    ).ap()

    with tile.TileContext(nc) as tc:
        with (
            tc.tile_pool(name="xt", bufs=1) as xt_pool,
            tc.tile_pool(name="w0", bufs=1) as w0_pool,
            tc.tile_pool(name="wr", bufs=2) as wr_pool,
            tc.tile_pool(name="y", bufs=8) as y_pool,
            tc.tile_pool(name="ps", bufs=1, space="PSUM") as ps_pool,
        ):
            warm = xt_pool.tile([P, 2], mybir.dt.float32, name="warm")
            # Warm the PE clock: HAM un-throttles (1.2->2.4GHz) only after
            # ~3.4us of sustained busy, so burn the ~11us DMA head on dummy
            # matmuls over never-written SBUF (values irrelevant; the real
            # groups start with start=True which resets the bank).
            dum_l = xt_pool.tile([P, P], mybir.dt.bfloat16, name="dum_l")
            dum_r = xt_pool.tile([P, NW], mybir.dt.bfloat16, name="dum_r")
            nc.vector.memset(dum_l[:], 0)
            nc.vector.memset(dum_r[:], 0)
            # Small-N dummies (~107ns cold) give fine-grained padding: they
            # keep the PE busy (HAM warm-up) from ~8.5us until the first
            # real operands land ~10.5us, with ~0.1us quantization.
            ps_warm = ps_pool.tile([P, NW], mybir.dt.float32, name="ps0_0")
            for _ in range(25):
                nc.tensor.matmul(
                    ps_warm[:, :64], lhsT=dum_l[:], rhs=dum_r[:, :64],
                    start=True, stop=True,
                )

            xb_sb = xt_pool.tile(
                [P, KTB, M_PER_CORE], mybir.dt.bfloat16, name="xb"
            )
            x8_sb = xt_pool.tile(
                [P, N8, 2, M_PER_CORE], mybir.dt.float8e4, name="x8"
            )
            # Head: interleave x-slice and w0 descriptors in consumption
            # order so MM(kt=0) waits on just the first two transfers.
            wb0_sbs = {}
            w80_sbs = {}
            for kind, j in MM_SEQ:
                if kind == "b":
                    if j == 0:
                        # kt0 goes out on the SCALAR queue: that engine's
                        # preamble ends ~2us before sync's, so the first
                        # operands land (and the PE starts) that much earlier.
                        # Emitted before the ACT-table warm-up below so the
                        # 1.3us table load doesn't delay the descriptors.
                        hm = M_PER_CORE // 2
                        nc.scalar.dma_start(xb_sb[:, 0, :hm], xb_d[0, :, :hm])
                        w_sb = w0_pool.tile(
                            [P, NW], mybir.dt.bfloat16, name="wb0_0"
                        )
                        nc.scalar.dma_start(w_sb[:], wb0_d[0])
                        nc.scalar.dma_start(xb_sb[:, 0, hm:], xb_d[0, :, hm:])
                        # warm the activation function table now (overlaps
                        # the head) so the tail's scalar.mul doesn't pay it
                        nc.scalar.mul(warm[:], warm[:], 0.0)
                    else:
                        nc.sync.dma_start(xb_sb[:, j, :], xb_d[j])
                        w_sb = w0_pool.tile(
                            [P, NW], mybir.dt.bfloat16, name=f"wb0_{j}"
                        )
                        nc.sync.dma_start(w_sb[:], wb0_d[j])
                    wb0_sbs[j] = w_sb
                else:
                    nc.sync.dma_start(x8_sb[:, j], x8_d[j])
                    w_sb = w0_pool.tile(
                        [P, 2, NW], mybir.dt.float8e4, name=f"w80_{j}"
                    )
                    nc.sync.dma_start(w_sb[:], w80_d[j])
                    w80_sbs[j] = w_sb
            # Prefetch stream for n-chunks 1-7 (pool slots throttle the
            # lookahead to ~1 chunk).
            wbr_sbs = {}
            w8r_sbs = {}
            for nch in range(1, NCH):
                for h in range(2):
                    w_sb = wr_pool.tile(
                        [P, GS, NW], mybir.dt.bfloat16, name=f"wbr{h}"
                    )
                    nc.sync.dma_start(w_sb[:], wbr_d[nch - 1, h])
                    wbr_sbs[(nch, h)] = w_sb
                w_sb = wr_pool.tile(
                    [P, N8, 2, NW], mybir.dt.float8e4, name="w8r"
                )
                nc.sync.dma_start(w_sb[:], w8r_d[nch - 1])
                w8r_sbs[nch] = w_sb

            def mm_group(ps, mt, wb_of_kt, w8_of_blk, seq):
                msl = slice(mt * P, (mt + 1) * P)
                for i, (kind, j) in enumerate(seq):
                    if kind == "b":
                        nc.tensor.matmul(
                            ps[:],
                            lhsT=xb_sb[:, j, msl],
                            rhs=wb_of_kt(j),
                            start=(i == 0),
                            stop=(i == len(seq) - 1),
                        )
                    else:
                        nc.tensor.matmul(
                            ps[:],
                            lhsT=x8_sb[:, j, :, msl],
                            rhs=w8_of_blk(j),
                            start=(i == 0),
                            stop=(i == len(seq) - 1),
                            perf_mode=mybir.MatmulPerfMode.DoubleRow,
                        )

            def emit_out(mt, nch, ps, last):
                """psum -> sbuf (descale by 2^-(a+b)) -> DRAM.  The last
                n-chunk spreads stores across both hw DMA queues (the input
                queue is idle by then) and the final group also splits the
                copy across engines, shrinking the serial tail."""
                y_sb = y_pool.tile([P, NW], mybir.dt.float32, name="y_sb")
                half = NW // 2
                if last:
                    nc.vector.tensor_scalar_mul(y_sb[:, :half], ps[:, :half], descale)
                    nc.scalar.mul(y_sb[:, half:], ps[:, half:], descale)
                else:
                    nc.vector.tensor_scalar_mul(y_sb[:], ps[:], descale)
                nq = 4 if last else 2
                step = NW // nq
                engs = (nc.scalar, nc.sync) if nch == NCH - 1 else (nc.scalar,)
                for s in range(nq):
                    engs[s % len(engs)].dma_start(
                        y_d[
                            mt * P : (mt + 1) * P,
                            nch * NW + s * step : nch * NW + (s + 1) * step,
                        ],
                        y_sb[:, s * step : (s + 1) * step],
                    )

            # n-chunk 0: k-outer over all 8 m-tiles (8 psum banks live) so
            # each w tile feeds 8 back-to-back MMs while the next streams in.
            ps0 = [
                ps_pool.tile([P, NW], mybir.dt.float32, name=f"ps0_{m}")
                for m in range(MT)
            ]
            for i, (kind, j) in enumerate(MM_SEQ):
                for mt in range(MT):
                    if kind == "b":
                        nc.tensor.matmul(
                            ps0[mt][:],
                            lhsT=xb_sb[:, j, mt * P : (mt + 1) * P],
                            rhs=wb0_sbs[j][:],
                            start=(i == 0),
                            stop=(i == len(MM_SEQ) - 1),
                        )
                    else:
                        nc.tensor.matmul(
                            ps0[mt][:],
                            lhsT=x8_sb[:, j, :, mt * P : (mt + 1) * P],
                            rhs=w80_sbs[j][:],
                            start=(i == 0),
                            stop=(i == len(MM_SEQ) - 1),
                            perf_mode=mybir.MatmulPerfMode.DoubleRow,
                        )
            for mt in range(MT):
                emit_out(mt, 0, ps0[mt], last=False)

            # n-chunks 1-7: m-inner (psum copies stagger across the sweep).
            gidx = 0
            for nch in range(1, NCH):
                for mt in range(MT):
                    seq = MM_SEQ[::-1] if gidx % 2 == 0 else MM_SEQ
                    gidx += 1
                    wb_f = lambda kt, n=nch: wbr_sbs[(n, kt // GS)][:, kt % GS, :]
                    w8_f = lambda blk, n=nch: w8r_sbs[n][:, blk]
                    if nch == NCH - 1 and mt == MT - 1:
                        # final group: two 256-col banks; bank a's output
                        # drains while bank b's MMs still run, leaving only
                        # a half-size copy+store after the last MM
                        half = NW // 2
                        ps_a = ps_pool.tile(
                            [P, half], mybir.dt.float32, name="ps0_6"
                        )
                        ps_b = ps_pool.tile(
                            [P, half], mybir.dt.float32, name=f"ps0_{mt}"
                        )
                        mm_group(
                            ps_a, mt,
                            lambda kt: wb_f(kt)[:, :half],
                            lambda blk: w8_f(blk)[:, :, :half],
                            seq,
                        )
                        y_a = y_pool.tile([P, half], mybir.dt.float32, name="y_sb")
                        nc.vector.tensor_scalar_mul(y_a[:], ps_a[:], descale)
                        for s, eng in ((0, nc.scalar), (1, nc.sync)):
                            q = half // 2
                            eng.dma_start(
                                y_d[
                                    mt * P : (mt + 1) * P,
                                    nch * NW + s * q : nch * NW + (s + 1) * q,
                                ],
                                y_a[:, s * q : (s + 1) * q],
                            )
                        mm_group(
                            ps_b, mt,
                            lambda kt: wb_f(kt)[:, half:],
                            lambda blk: w8_f(blk)[:, :, half:],
                            seq[::-1],
                        )
                        y_b = y_pool.tile([P, half], mybir.dt.float32, name="y_sb")
                        q = half // 2
                        nc.vector.tensor_scalar_mul(y_b[:, :q], ps_b[:, :q], descale)
                        nc.scalar.mul(y_b[:, q:], ps_b[:, q:], descale)
                        for s, eng in ((0, nc.scalar), (1, nc.sync)):
                            eng.dma_start(
                                y_d[
                                    mt * P : (mt + 1) * P,
                                    nch * NW + half + s * q : nch * NW + half + (s + 1) * q,
                                ],
                                y_b[:, s * q : (s + 1) * q],
                            )
                    else:
                        ps = ps_pool.tile(
                            [P, NW], mybir.dt.float32, name=f"ps0_{mt}"
                        )
                        mm_group(ps, mt, wb_f, w8_f, seq)
                        emit_out(mt, nch, ps, last=False)
    _BUILT[descale] = nc
    return nc


def _ensure_ntff_hook():
    """bass_utils' trace path imports antenv.axon_hooks, which some images
    lack (trn_boot degrades silently).  Recreate the glue module around the
    libaxon_pjrt.so ctypes hook so trace=True works; no-op if present."""
    import sys
    import types

    try:
        import antenv.axon_hooks  # noqa: F401

        return
    except ImportError:
        pass
    try:
        import antenv

        if "/root/.axon_site" not in sys.path:
            sys.path.insert(0, "/root/.axon_site")
        from trn_agent_boot.trn_boot import _ntff_profile_via_ctypes

        hook = _ntff_profile_via_ctypes("/opt/axon/libaxon_pjrt.so")
        mod = types.ModuleType("antenv.axon_hooks")
        mod._hook = hook
        mod.get_axon_ntff_profile_hook = lambda: mod._hook
        mod.set_axon_ntff_profile_hook = lambda h: setattr(mod, "_hook", h)
        sys.modules["antenv.axon_hooks"] = mod
        antenv.axon_hooks = mod
    except Exception:
        pass  # trace attempt will fall back to trace=False below


def kernel(x, w_q_fp4, w_os_fp4, w_is_fp4, w_t, w_q_fp8, w_s_fp8):
    global LAST_RESULT
    from concourse.bass_utils import run_bass_kernel_spmd

    x = np.asarray(x, dtype=np.float32)
    w_t = np.asarray(w_t, dtype=np.float32)

    bf16 = ml_dtypes.bfloat16
    e4m3 = ml_dtypes.float8_e4m3  # TRN fp8e4: max normal 240

    # power-2 scales placing the fp8-span operands in e4m3 range
    a = float(np.floor(np.log2(FP8_MAX / np.abs(x).max())))
    b = float(np.floor(np.log2(FP8_MAX / np.abs(w_t[KSPLIT:, :]).max())))
    sa, sb = 2.0**a, 2.0**b
    descale = float(2.0 ** (-(a + b)))

    nc = _build(descale)

    def to8(v, s):
        return np.clip(v * s, -FP8_MAX, FP8_MAX).astype(e4m3)

    xt = np.ascontiguousarray(x.T)                     # [IN, TOKENS] fp32
    xb_all = (xt[:KSPLIT] * sa).astype(bf16)           # [KSPLIT, TOKENS]
    x8_all = to8(xt[KSPLIT:], sa)                      # [2*N8*P, TOKENS]

    wsc = w_t * sb
    # n-chunk 0
    wb0 = np.ascontiguousarray(wsc[:KSPLIT, :NW]).astype(bf16).reshape(KTB, P, NW)
    w80 = np.ascontiguousarray(
        to8(wsc[KSPLIT:, :NW], 1.0).reshape(N8, 2, P, NW).transpose(0, 2, 1, 3)
    )
    # n-chunks 1-7
    wbr = np.ascontiguousarray(
        wsc[:KSPLIT, NW:]
        .astype(bf16)
        .reshape(2, GS, P, NCH - 1, NW)
        .transpose(3, 0, 2, 1, 4)
    )
    w8r = np.ascontiguousarray(
        to8(wsc[KSPLIT:, NW:], 1.0)
        .reshape(N8, 2, P, NCH - 1, NW)
        .transpose(3, 2, 0, 1, 4)
    )
    in_maps = []
    for i in range(NCORES):
        msl = slice(i * M_PER_CORE, (i + 1) * M_PER_CORE)
        xb = np.ascontiguousarray(xb_all[:, msl]).reshape(KTB, P, M_PER_CORE)
        x8 = np.ascontiguousarray(
            x8_all[:, msl].reshape(N8, 2, P, M_PER_CORE).transpose(0, 2, 1, 3)
        )
        in_maps.append(
            {"xb": xb, "x8": x8, "wb0": wb0, "w80": w80, "wbr": wbr, "w8r": w8r}
        )
    want_trace = bool(os.environ.get("BASS_TRACE"))
    if want_trace:
        _ensure_ntff_hook()
    res = None
    # retries cover transient device errors (e.g. NRT_EXEC_UNIT_UNRECOVERABLE,
    # observed once and succeeded on retry); the final attempt drops trace in
    # case the profiling path itself is what broke
    for attempt, tr in enumerate((want_trace, want_trace, False)):
        try:
            res = run_bass_kernel_spmd(nc, in_maps, list(range(NCORES)), trace=tr)
            break
        except Exception:
            if attempt == 2:
                raise
    LAST_RESULT = res
    return np.concatenate([res.results[i]["y"] for i in range(NCORES)], axis=0)


# revision 27
# speedup vs baseline: 1.0097x; 1.0097x over previous
"""Trainium2 Bass kernel for nn_MixedLinear_89979564851799.

The reference computes y = x @ W.T where W is the block-dequantized weight;
setup_inputs() ships the module's precomputed dequantized transposed weight
w_t (IN, OUT), so y == x @ w_t up to fp32 matmul reassociation.  The kernel
runs a single 8192x4096x4096 matmul, data-parallel over tokens across 8
NeuronCores.

Numerics: mixed bf16 / fp8-DoubleRow.  The last K8 = 256*N8 of the
contraction runs as fp8e4 DoubleRow matmuls (2 k-tiles per MM at the same
216ns issue gap as one bf16 MM -> 2x throughput on that span; measured
with probe_doublerow.py on this part).  The fp8 range covers the module's
fp8-quantized weight partition (k in [3584,4096), whose dequantized values
are EXACTLY representable in TRN fp8e4 under a power-2 scale) plus
256*(N8-2) columns of the fp4 partition (e4m3 rounding error ~2.4% rms on
that slice).  x is e4m3 on the fp8 range.  CPU simulation of the exact
scheme on the reference data: rel err 9.6e-3 (N8=2) / 1.34e-2 (N8=3) /
1.63e-2 (N8=4) vs the 2e-2 gate; bf16-only measures 2.26e-3.  Measured HW
rel err matches the simulation to 5 digits (1.632e-2 at N8=4).  N8=5
(1.88e-2) leaves too little margin even with per-column scale tricks.

Scale handling: fp8 operands need power-2 scaling (x*2^a, w*2^b) to sit in
e4m3 range; the bf16 operands are pre-scaled by the same powers (exact in
bf16) so both matmul flavors accumulate into one PSUM group, and the
psum->sbuf copy applies 2^-(a+b) (tensor_scalar_mul, same cost as the
plain copy).

Schedule (from ntff profiles of each round): interleaved per-k-tile DMA
descriptors in consumption order; n-chunk 0 k-OUTER across 8 m-tiles / 8
psum banks (PE starts ~11us in and never starves: demand 222GB/s < the
~300GB/s 16 DMA engines deliver); n-chunks 1-7 m-inner with staggered psum
copies; dummy N=64 matmuls pad the DMA head so the PE's HAM clock gate is
warm (2.4GHz) when real work starts; activation table warmed early; the
last n-chunk's stores spread across both hw DMA queues and the final
group's copy split across DVE+ACT to shrink the serial tail.

History on this part: baseline bf16 469.5us -> DMA/schedule fixes 464.3
-> +fp8 DoubleRow 408.1 -> head/tail tuning ~405.2us (PE roofline for the
1792-MM stream is ~387us; the rest is fixed preamble ~7.5us, DMA head,
~7us of LDWEIGHTS-reorder hiccups, and the drain tail).
"""

import os
import numpy as np
import ml_dtypes

P = 128
TOKENS, IN, OUT = 8192, 4096, 4096
NCORES = 8
M_PER_CORE = TOKENS // NCORES      # 1024
KT = IN // P                       # 32 k-tiles
MT = M_PER_CORE // P               # 8 m-tiles
NCH = 8                            # n chunks
NW = OUT // NCH                    # 512 cols per chunk (= 1 PSUM bank fp32)

N8 = 4                             # DoubleRow 256-k blocks (fp8 span = 256*N8)
KTB = KT - 2 * N8                  # bf16 k-tiles
KSPLIT = KTB * P                   # k index where the fp8 span starts
GS = KTB // 2                      # bf16 w chunk size (k-tiles) for nch 1-7

FP8_MAX = 240.0                    # TRN fp8e4 max normal

# Per-group matmul emission order.  Toggling DoubleRow mode costs ~190ns
# (measured: fully interleaving DR among bf16 MMs added ~84us = ~8
# transitions/group), so DR blocks stay clustered at one end, and the
# m-inner groups snake (alternate groups run DR-first) so consecutive
# groups' DR clusters abut: ~1 mode transition per group instead of 2.
MM_SEQ = [("b", _kt) for _kt in range(KTB)] + [("d", _j) for _j in range(N8)]

# Results of the traced run (exec_time_ns etc.) for test harnesses.
LAST_RESULT = None
_BUILT = {}


def _patch_tile_drain():
    """The walrus build in this container rejects instructions carrying more
    than one sync-wait (CoreV3GenImpl setupSyncWait: "Too many sync wait
    commands").  Tile's scheduler freely assigns several waits to one
    instruction, so (a) wrap _commit_instruction to hoist extra waits onto
    single-wait NOPs on the same engine just before the offender, and
    (b) split the kernel-tail Drain (which collects one wait per DMA queue)
    into a chain of single-wait Drains."""
    import concourse.tile as tile_mod
    import concourse.mybir as mybir
    import bass_rust
    from concourse.vector_clock import ScopedClock

    if getattr(tile_mod.TileContext, "_single_wait_drain_patch", False):
        return

    orig_commit = tile_mod.TileContext._commit_instruction

    def _commit_instruction(self, inst, lazy_reg_writes=True):
        si = getattr(inst, "sync_info", None)
        if (
            si is not None
            and len(si.on_wait) > 1
            and inst.engine != mybir.EngineType.Unassigned
        ):
            waits = list(si.on_wait)
            for w in waits[:-1]:
                nop = mybir.InstNoOp(
                    name=self.nc.get_next_instruction_name(),
                    engine=inst.engine,
                    sync_info=mybir.SyncInfo(on_wait=[w], on_update=[]),
                    bass_nofuse=True,
                )
                orig_commit(self, nop, lazy_reg_writes=False)
            inst.sync_info = mybir.SyncInfo(
                on_wait=[waits[-1]], on_update=list(si.on_update)
            )
        return orig_commit(self, inst, lazy_reg_writes)

    tile_mod.TileContext._commit_instruction = _commit_instruction

    def _drain_and_barrier(self, tick_clock, wait_clock):
        drain_inst = self.nc.sync.drain()
        wait_clock.add_sem_waits(
            drain_inst.ins, ScopedClock({None: tick_clock.global_clock})
        )
        si = drain_inst.ins.sync_info
        if si is not None and len(si.on_wait) > 1:
            waits = list(si.on_wait)
            drain_inst.ins.sync_info = bass_rust.SyncInfo(
                on_wait=[waits[0]], on_update=list(si.on_update)
            )
            for w in waits[1:]:
                extra = self.nc.sync.drain()
                extra.ins.sync_info = bass_rust.SyncInfo(on_wait=[w], on_update=[])
        self.nc.all_engine_barrier()
        popped = self.nc._tile_sem_poison_stack.pop()
        assert popped is self._sem_poison
        self.nc.clear_and_free_semaphores(list(self.sems.allocated().values()))
        self.nc.all_engine_barrier()

    tile_mod.TileContext._drain_and_barrier = _drain_and_barrier
    tile_mod.TileContext._single_wait_drain_patch = True


def _build(descale):
    """descale = 2^-(a+b), baked into the psum->sbuf copies."""
    if descale in _BUILT:
        return _BUILT[descale]
    import concourse.bass as bass
    import concourse.tile as tile
    from concourse import mybir

    _patch_tile_drain()

    nc = bass.Bass("TRN2", debug=False)
    xb_d = nc.dram_tensor(
        "xb", [KTB, P, M_PER_CORE], mybir.dt.bfloat16, kind="ExternalInput"
    ).ap()
    x8_d = nc.dram_tensor(
        "x8", [N8, P, 2, M_PER_CORE], mybir.dt.float8e4, kind="ExternalInput"
    ).ap()
    # n-chunk 0 of w, per-k-tile descriptors
    wb0_d = nc.dram_tensor(
        "wb0", [KTB, P, NW], mybir.dt.bfloat16, kind="ExternalInput"
    ).ap()
    w80_d = nc.dram_tensor(
        "w80", [N8, P, 2, NW], mybir.dt.float8e4, kind="ExternalInput"
    ).ap()
    # n-chunks 1-7: bf16 in two GS-k-tile chunks, fp8 in one block
    wbr_d = nc.dram_tensor(
        "wbr", [NCH - 1, 2, P, GS, NW], mybir.dt.bfloat16, kind="ExternalInput"
    ).ap()
    w8r_d = nc.dram_tensor(
        "w8r", [NCH - 1, P, N8, 2, NW], mybir.dt.float8e4, kind="ExternalInput"
    ).ap()
    y_d = nc.dram_tensor(
        "y", [M_PER_CORE, OUT], mybir.dt.float32, kind="ExternalOutput"
    ).ap()

    with tile.TileContext(nc) as tc:
        with (
            tc.tile_pool(name="xt", bufs=1) as xt_pool,
            tc.tile_pool(name="w0", bufs=1) as w0_pool,
            tc.tile_pool(name="wr", bufs=2) as wr_pool,
            tc.tile_pool(name="y", bufs=8) as y_pool,
            tc.tile_pool(name="ps", bufs=1, space="PSUM") as ps_pool,
        ):
            # Warm the activation engine's function table (1.3us, overlaps
            # the DMA head) so the tail's scalar.mul doesn't pay it.
            warm = xt_pool.tile([P, 2], mybir.dt.float32, name="warm")
            nc.scalar.mul(warm[:], warm[:], 0.0)
            # Warm the PE clock: HAM un-throttles (1.2->2.4GHz) only after
            # ~3.4us of sustained busy, so burn the ~11us DMA head on dummy
            # matmuls over never-written SBUF (values irrelevant; the real
            # groups start with start=True which resets the bank).
            dum_l = xt_pool.tile([P, P], mybir.dt.bfloat16, name="dum_l")
            dum_r = xt_pool.tile([P, NW], mybir.dt.bfloat16, name="dum_r")
            nc.vector.memset(dum_l[:], 0)
            nc.vector.memset(dum_r[:], 0)
            # Small-N dummies (~107ns cold) give fine-grained padding: they
            # keep the PE busy (HAM warm-up) from ~8.5us until the first
            # real operands land ~10.5us, with ~0.1us quantization.
            ps_warm = ps_pool.tile([P, NW], mybir.dt.float32, name="ps0_0")
            for _ in range(60):
                nc.tensor.matmul(
                    ps_warm[:, :64], lhsT=dum_l[:], rhs=dum_r[:, :64],
                    start=True, stop=True,
                )

            xb_sb = xt_pool.tile(
                [P, KTB, M_PER_CORE], mybir.dt.bfloat16, name="xb"
            )
            x8_sb = xt_pool.tile(
                [P, N8, 2, M_PER_CORE], mybir.dt.float8e4, name="x8"
            )
            # Head: interleave x-slice and w0 descriptors in consumption
            # order so MM(kt=0) waits on just the first two transfers.
            wb0_sbs = {}
            w80_sbs = {}
            for kind, j in MM_SEQ:
                if kind == "b":
                    if j == 0:
                        # split so the first MM (kt0, m0) waits on 128KB
                        hm = M_PER_CORE // 2
                        nc.sync.dma_start(xb_sb[:, 0, :hm], xb_d[0, :, :hm])
                        w_sb = w0_pool.tile(
                            [P, NW], mybir.dt.bfloat16, name="wb0_0"
                        )
                        nc.sync.dma_start(w_sb[:], wb0_d[0])
                        nc.sync.dma_start(xb_sb[:, 0, hm:], xb_d[0, :, hm:])
                    else:
                        nc.sync.dma_start(xb_sb[:, j, :], xb_d[j])
                        w_sb = w0_pool.tile(
                            [P, NW], mybir.dt.bfloat16, name=f"wb0_{j}"
                        )
                        nc.sync.dma_start(w_sb[:], wb0_d[j])
                    wb0_sbs[j] = w_sb
                else:
                    nc.sync.dma_start(x8_sb[:, j], x8_d[j])
                    w_sb = w0_pool.tile(
                        [P, 2, NW], mybir.dt.float8e4, name=f"w80_{j}"
                    )
                    nc.sync.dma_start(w_sb[:], w80_d[j])
                    w80_sbs[j] = w_sb
            # Prefetch stream for n-chunks 1-7 (pool slots throttle the
            # lookahead to ~1 chunk).
            wbr_sbs = {}
            w8r_sbs = {}
            for nch in range(1, NCH):
                for h in range(2):
                    w_sb = wr_pool.tile(
                        [P, GS, NW], mybir.dt.bfloat16, name=f"wbr{h}"
                    )
                    nc.sync.dma_start(w_sb[:], wbr_d[nch - 1, h])
                    wbr_sbs[(nch, h)] = w_sb
                w_sb = wr_pool.tile(
                    [P, N8, 2, NW], mybir.dt.float8e4, name="w8r"
                )
                nc.sync.dma_start(w_sb[:], w8r_d[nch - 1])
                w8r_sbs[nch] = w_sb

            def mm_group(ps, mt, wb_of_kt, w8_of_blk, seq):
                msl = slice(mt * P, (mt + 1) * P)
                for i, (kind, j) in enumerate(seq):
                    if kind == "b":
                        nc.tensor.matmul(
                            ps[:],
                            lhsT=xb_sb[:, j, msl],
                            rhs=wb_of_kt(j),
                            start=(i == 0),
                            stop=(i == len(seq) - 1),
                        )
                    else:
                        nc.tensor.matmul(
                            ps[:],
                            lhsT=x8_sb[:, j, :, msl],
                            rhs=w8_of_blk(j),
                            start=(i == 0),
                            stop=(i == len(seq) - 1),
                            perf_mode=mybir.MatmulPerfMode.DoubleRow,
                        )

            def emit_out(mt, nch, ps, last):
                """psum -> sbuf (descale by 2^-(a+b)) -> DRAM.  The last
                n-chunk spreads stores across both hw DMA queues (the input
                queue is idle by then) and the final group also splits the
                copy across engines, shrinking the serial tail."""
                y_sb = y_pool.tile([P, NW], mybir.dt.float32, name="y_sb")
                half = NW // 2
                if last:
                    nc.vector.tensor_scalar_mul(y_sb[:, :half], ps[:, :half], descale)
                    nc.scalar.mul(y_sb[:, half:], ps[:, half:], descale)
                else:
                    nc.vector.tensor_scalar_mul(y_sb[:], ps[:], descale)
                nq = 4 if last else 2
                step = NW // nq
                engs = (nc.scalar, nc.sync) if nch == NCH - 1 else (nc.scalar,)
                for s in range(nq):
                    engs[s % len(engs)].dma_start(
                        y_d[
                            mt * P : (mt + 1) * P,
                            nch * NW + s * step : nch * NW + (s + 1) * step,
                        ],
                        y_sb[:, s * step : (s + 1) * step],
                    )

            # n-chunk 0: k-outer over all 8 m-tiles (8 psum banks live) so
            # each w tile feeds 8 back-to-back MMs while the next streams in.
            ps0 = [
                ps_pool.tile([P, NW], mybir.dt.float32, name=f"ps0_{m}")
                for m in range(MT)
            ]
            for i, (kind, j) in enumerate(MM_SEQ):
                for mt in range(MT):
                    if kind == "b":
                        nc.tensor.matmul(
                            ps0[mt][:],
                            lhsT=xb_sb[:, j, mt * P : (mt + 1) * P],
                            rhs=wb0_sbs[j][:],
                            start=(i == 0),
                            stop=(i == len(MM_SEQ) - 1),
                        )
                    else:
                        nc.tensor.matmul(
                            ps0[mt][:],
                            lhsT=x8_sb[:, j, :, mt * P : (mt + 1) * P],
                            rhs=w80_sbs[j][:],
                            start=(i == 0),
                            stop=(i == len(MM_SEQ) - 1),
                            perf_mode=mybir.MatmulPerfMode.DoubleRow,
                        )
            for mt in range(MT):
                emit_out(mt, 0, ps0[mt], last=False)

            # n-chunks 1-7: m-inner (psum copies stagger across the sweep).
            gidx = 0
            for nch in range(1, NCH):
                for mt in range(MT):
                    ps = ps_pool.tile([P, NW], mybir.dt.float32, name=f"ps0_{mt}")
                    seq = MM_SEQ[::-1] if gidx % 2 == 0 else MM_SEQ
                    gidx += 1
                    mm_group(
                        ps,
                        mt,
                        lambda kt, n=nch: wbr_sbs[(n, kt // GS)][:, kt % GS, :],
                        lambda blk, n=nch: w8r_sbs[n][:, blk],
                        seq,
                    )
                    emit_out(
                        mt, nch, ps, last=(nch == NCH - 1 and mt == MT - 1)
                    )
    _BUILT[descale] = nc
    return nc


def _ensure_ntff_hook():
    """bass_utils' trace path imports antenv.axon_hooks, which some images
    lack (trn_boot degrades silently).  Recreate the glue module around the
    libaxon_pjrt.so ctypes hook so trace=True works; no-op if present."""
    import sys
    import types

    try:
        import antenv.axon_hooks  # noqa: F401

        return
    except ImportError:
        pass
    try:
        import antenv

        if "/root/.axon_site" not in sys.path:
            sys.path.insert(0, "/root/.axon_site")
        from trn_agent_boot.trn_boot import _ntff_profile_via_ctypes

        hook = _ntff_profile_via_ctypes("/opt/axon/libaxon_pjrt.so")
        mod = types.ModuleType("antenv.axon_hooks")
        mod._hook = hook
        mod.get_axon_ntff_profile_hook = lambda: mod._hook
        mod.set_axon_ntff_profile_hook = lambda h: setattr(mod, "_hook", h)
        sys.modules["antenv.axon_hooks"] = mod
        antenv.axon_hooks = mod
    except Exception:
        pass  # trace attempt will fall back to trace=False below


def kernel(x, w_q_fp4, w_os_fp4, w_is_fp4, w_t, w_q_fp8, w_s_fp8):
    global LAST_RESULT
    from concourse.bass_utils import run_bass_kernel_spmd

    x = np.asarray(x, dtype=np.float32)
    w_t = np.asarray(w_t, dtype=np.float32)

    bf16 = ml_dtypes.bfloat16
    e4m3 = ml_dtypes.float8_e4m3  # TRN fp8e4: max normal 240

    # power-2 scales placing the fp8-span operands in e4m3 range
    a = float(np.floor(np.log2(FP8_MAX / np.abs(x).max())))
    b = float(np.floor(np.log2(FP8_MAX / np.abs(w_t[KSPLIT:, :]).max())))
    sa, sb = 2.0**a, 2.0**b
    descale = float(2.0 ** (-(a + b)))

    nc = _build(descale)

    def to8(v, s):
        return np.clip(v * s, -FP8_MAX, FP8_MAX).astype(e4m3)

    xt = np.ascontiguousarray(x.T)                     # [IN, TOKENS] fp32
    xb_all = (xt[:KSPLIT] * sa).astype(bf16)           # [KSPLIT, TOKENS]
    x8_all = to8(xt[KSPLIT:], sa)                      # [2*N8*P, TOKENS]

    wsc = w_t * sb
    # n-chunk 0
    wb0 = np.ascontiguousarray(wsc[:KSPLIT, :NW]).astype(bf16).reshape(KTB, P, NW)
    w80 = np.ascontiguousarray(
        to8(wsc[KSPLIT:, :NW], 1.0).reshape(N8, 2, P, NW).transpose(0, 2, 1, 3)
    )
    # n-chunks 1-7
    wbr = np.ascontiguousarray(
        wsc[:KSPLIT, NW:]
        .astype(bf16)
        .reshape(2, GS, P, NCH - 1, NW)
        .transpose(3, 0, 2, 1, 4)
    )
    w8r = np.ascontiguousarray(
        to8(wsc[KSPLIT:, NW:], 1.0)
        .reshape(N8, 2, P, NCH - 1, NW)
        .transpose(3, 2, 0, 1, 4)
    )
    in_maps = []
    for i in range(NCORES):
        msl = slice(i * M_PER_CORE, (i + 1) * M_PER_CORE)
        xb = np.ascontiguousarray(xb_all[:, msl]).reshape(KTB, P, M_PER_CORE)
        x8 = np.ascontiguousarray(
            x8_all[:, msl].reshape(N8, 2, P, M_PER_CORE).transpose(0, 2, 1, 3)
        )
        in_maps.append(
            {"xb": xb, "x8": x8, "wb0": wb0, "w80": w80, "wbr": wbr, "w8r": w8r}
        )
    want_trace = bool(os.environ.get("BASS_TRACE"))
    if want_trace:
        _ensure_ntff_hook()
    res = None
    # retries cover transient device errors (e.g. NRT_EXEC_UNIT_UNRECOVERABLE,
    # observed once and succeeded on retry); the final attempt drops trace in
    # case the profiling path itself is what broke
    for attempt, tr in enumerate((want_trace, want_trace, False)):
        try:
            res = run_bass_kernel_spmd(nc, in_maps, list(range(NCORES)), trace=tr)
            break
        except Exception:
            if attempt == 2:
                raise
    LAST_RESULT = res
    return np.concatenate([res.results[i]["y"] for i in range(NCORES)], axis=0)
